# revision 1
# baseline (speedup 1.0000x reference)
"""Trainium2 Bass kernel for the 2-layer GAT model (top-10 attention, 4 heads).

Strategy (8 NeuronCores, SPMD):
- Nodes sharded into 8 contiguous ranges of 6250 (dst ranges == GEMM shards).
  Random dst => edge counts per range are balanced to ~1%.
- Within each core, dst nodes are sorted by degree (host-side permutation) and
  grouped into 49 tiles of 128 with a common per-tile slot count D[t]
  (shared across cores so one NEFF serves all 8 cores).
- Feature transform xl = x @ W.T runs as a distributed GEMM over the shard
  (PE-transpose of input tiles + matmul against host-pretransposed weights,
  with the attention projections a_s/a_d folded in as 8 extra columns);
  shards are AllGathered into a replicated table per core.
- Edge phase per tile: gather per-slot a_s (dma_gather, 256B rows), build
  alpha = leaky_relu(a_s+a_d), mask host-known pad slots, top-10 per
  (dst, head) via DVE max8 + match_replace + max8 (threshold = 10th largest),
  ex = exp(alpha - max) * masks, denom via row reduce. Then stream the 1KB
  xl[src] rows with dma_gather in j-major slot order (signed-int16 index
  trick: base at row 32768 covers 50176 rows), multiply by ex in-place and
  accumulate per slot column with an identity matmul into PSUM.
- Layer 2 repeats with W2 on layer-1 output; head-mean + 2-layer MLP fused
  per tile; results unpermuted on host.
"""
import numpy as np

N = 50000
E = 800000
F_IN = 256
H, C = 4, 64
HC = H * C
K_TOP = 10
NEG_SLOPE = 0.2
N_CORES = 8
SH = N // N_CORES            # 6250 real rows per core
TILES = (SH + 127) // 128    # 49
ROWS = TILES * 128           # 6272 padded rows per core
NT = N_CORES * ROWS          # 50176 global table rows
BASE = 32768                 # signed-int16 gather base row
PADROW = NT - 1              # ghost row of last core (content = f(zeros))
JC = 16                      # xg gather chunk (j columns per dma_gather)
HID, OUT_F = 128, 16


def _wrap_idx(vals: np.ndarray) -> np.ndarray:
    """int16 index list -> [128, ceil(len/16)] wrapped+replicated tile."""
    ni = len(vals)
    w = -(-ni // 16)
    arr = np.full(w * 16, PADROW - BASE, np.int16)
    arr[:ni] = vals
    return np.tile(arr.reshape(w, 16).T, (8, 1))


def _prep(x, W1, att_s1, att_d1, W2, att_s2, att_d2, Wl1, Wl2, edge_index):
    """Host preprocessing: sharding, degree-sorted tiles, gather index tables."""
    src = np.asarray(edge_index[0], np.int64)
    dst = np.asarray(edge_index[1], np.int64)

    deg = np.bincount(dst, minlength=N)
    # per-core degree-sorted local ordering
    loc = np.empty(N, np.int64)
    node_of = np.full((N_CORES, ROWS), -1, np.int64)  # local row -> global node
    for c in range(N_CORES):
        nodes = np.arange(c * SH, (c + 1) * SH)
        order = np.argsort(-deg[nodes], kind="stable")
        loc[nodes[order]] = np.arange(SH)
        node_of[c, :SH] = nodes[order]
    rowid = (dst // SH) * ROWS + 0  # placeholder; rowid is for any node:
    rowid = (np.arange(N) // SH) * ROWS + loc  # node -> global table row

    # common D schedule
    degl = np.zeros((N_CORES, ROWS), np.int64)
    for c in range(N_CORES):
        degl[c, :SH] = deg[node_of[c, :SH]]
    tile_max = degl.reshape(N_CORES, TILES, 128).max(axis=(0, 2))
    D = np.maximum(8, ((tile_max + 3) // 4) * 4).astype(np.int64)
    assert D.max() <= 120, f"degree too high for this kernel: {D.max()}"

    # CSR of edges by (core, local dst row)
    e_loc = (dst // SH) * ROWS + loc[dst]     # global permuted row of each dst
    order_e = np.argsort(e_loc, kind="stable")
    src_s = src[order_e]
    e_loc_s = e_loc[order_e]
    starts = np.searchsorted(e_loc_s, np.arange(N_CORES * ROWS))
    ends = np.searchsorted(e_loc_s, np.arange(N_CORES * ROWS) + 1)

    # slot tables: per core, per tile: [128, D[t]] of global rowids (or PADROW)
    PAD16 = np.int16(PADROW - BASE)
    idx_as_parts = [[] for _ in range(N_CORES)]
    idx_xg_parts = [[] for _ in range(N_CORES)]
    chunks = []  # per tile: list of (j0, jc)
    for t in range(TILES):
        Dt = int(D[t])
        ch = [(j0, min(JC, Dt - j0)) for j0 in range(0, Dt, JC)]
        chunks.append(ch)
    for c in range(N_CORES):
        for t in range(TILES):
            Dt = int(D[t])
            slot = np.full((128, Dt), PADROW, np.int64)
            for d in range(128):
                r = c * ROWS + t * 128 + d
                s, e = starts[r], ends[r]
                if e > s:
                    slot[d, : e - s] = rowid[src_s[s:e]]
            s16 = (slot - BASE).astype(np.int16)
            # a_s gather: j-major full tile + guard
            jm = s16.T.reshape(-1)  # [D*128] j-major
            idx_as_parts[c].append(_wrap_idx(np.concatenate([jm, [PAD16]])))
            # xg gathers: per chunk
            for (j0, jc) in chunks[t]:
                part = s16[:, j0 : j0 + jc].T.reshape(-1)
                idx_xg_parts[c].append(_wrap_idx(np.concatenate([part, [PAD16]])))

    idx_as = np.stack([np.concatenate(p, axis=1) for p in idx_as_parts])
    idx_xg = np.stack([np.concatenate(p, axis=1) for p in idx_xg_parts])

    degf = np.zeros((N_CORES, 128, TILES), np.float32)
    for c in range(N_CORES):
        degf[c] = degl[c].reshape(TILES, 128).T.astype(np.float32)

    x_shard = np.zeros((N_CORES, ROWS, F_IN), np.float32)
    for c in range(N_CORES):
        x_shard[c, :SH] = np.asarray(x)[node_of[c, :SH]]

    def att_fold(WT, att_s, att_d):
        Vs = np.stack([WT[:, h * C : (h + 1) * C] @ np.asarray(att_s)[0, h]
                       for h in range(H)], axis=1)
        Vd = np.stack([WT[:, h * C : (h + 1) * C] @ np.asarray(att_d)[0, h]
                       for h in range(H)], axis=1)
        return np.hstack([WT, Vs, Vd]).astype(np.float32)

    W1T_ext = att_fold(np.asarray(W1).T.astype(np.float32), att_s1, att_d1)
    W2T_ext = att_fold(np.asarray(W2).T.astype(np.float32), att_s2, att_d2)

    meta = dict(D=[int(d) for d in D], chunks=chunks)
    consts = dict(
        W1T_ext=W1T_ext, W2T_ext=W2T_ext,
        Wl1T=np.asarray(Wl1).T.astype(np.float32).copy(),
        Wl2T=np.asarray(Wl2).T.astype(np.float32).copy(),
    )
    per_core = dict(x_shard=x_shard, idx_as=idx_as, idx_xg=idx_xg, degf=degf)
    return meta, consts, per_core, node_of


def build_gnn(meta, stage=4, sub=99, repeat=1):
    from concourse import bass, bacc, mybir
    import concourse.tile as tile
    from concourse.masks import make_identity

    D = meta["D"]
    chunks = meta["chunks"]
    WAS = sum(-(-(128 * D[t] + 1) // 16) for t in range(TILES))
    WXG = sum(-(-(128 * jc + 1) // 16) for t in range(TILES) for (_, jc) in chunks[t])

    f32 = mybir.dt.float32
    i16 = mybir.dt.int16
    nc = bacc.Bacc(None, target_bir_lowering=False, num_devices=N_CORES,
                   num_swdge_queues=4)

    # inputs
    x_in = nc.dram_tensor("x_shard", [ROWS, F_IN], f32, kind="ExternalInput")
    w1_in = nc.dram_tensor("W1T_ext", [F_IN, HC + 8], f32, kind="ExternalInput")
    w2_in = nc.dram_tensor("W2T_ext", [HC, HC + 8], f32, kind="ExternalInput")
    wl1_in = nc.dram_tensor("Wl1T", [C, HID], f32, kind="ExternalInput")
    wl2_in = nc.dram_tensor("Wl2T", [HID, OUT_F], f32, kind="ExternalInput")
    bl1_in = nc.dram_tensor("bl1_col", [HID, 1], f32, kind="ExternalInput")
    bl2_in = nc.dram_tensor("bl2_rep", [128, OUT_F], f32, kind="ExternalInput")
    b1_in = nc.dram_tensor("b1_rep", [128, HC], f32, kind="ExternalInput")
    b2_in = nc.dram_tensor("b2_rep", [128, C], f32, kind="ExternalInput")
    dg_in = nc.dram_tensor("degf", [128, TILES], f32, kind="ExternalInput")
    ia_in = nc.dram_tensor("idx_as", [128, WAS], i16, kind="ExternalInput")
    ix_in = nc.dram_tensor("idx_xg", [128, WXG], i16, kind="ExternalInput")

    out_dram = nc.dram_tensor("out", [ROWS, OUT_F], f32, kind="ExternalOutput")

    # internal DRAM
    xl_sh = [nc.dram_tensor(f"xl{l}_shard", [ROWS, HC], f32) for l in (1, 2)]
    asd_lo = [nc.dram_tensor(f"asd{l}_local", [ROWS, 8], f32) for l in (1, 2)]
    xl_fu = [nc.dram_tensor(f"xl{l}_full", [NT, HC], f32, addr_space="Shared")
             for l in (1, 2)]
    asd_fu = [nc.dram_tensor(f"asd{l}_full", [NT, 8], f32, addr_space="Shared")
              for l in (1, 2)]
    asd_pad = [nc.dram_tensor(f"asd{l}_pad", [NT, 64], f32) for l in (1, 2)]
    out1_dram = nc.dram_tensor("out1_dram", [ROWS, HC], f32)

    LR = mybir.ActivationFunctionType.Lrelu
    CPY = mybir.ActivationFunctionType.Copy
    MAXO = mybir.AluOpType.max
    EXP = mybir.ActivationFunctionType.Exp
    RELU = mybir.ActivationFunctionType.Relu
    ADD = mybir.AluOpType.add
    MUL = mybir.AluOpType.mult
    SUB = mybir.AluOpType.subtract
    GE = mybir.AluOpType.is_ge
    LT = mybir.AluOpType.is_lt

    with tile.TileContext(nc) as tc:
        with (
            tc.tile_pool(name="const", bufs=1) as cpool,
            tc.tile_pool(name="gemm", bufs=6) as gpool,
            tc.tile_pool(name="gpsum", bufs=2, space="PSUM") as gpsum,
            tc.tile_pool(name="edge", bufs=4) as epool,
            tc.tile_pool(name="small", bufs=8) as spool,
            tc.tile_pool(name="agg", bufs=2, space="PSUM") as apsum,
            tc.tile_pool(name="mlpp", bufs=1, space="PSUM") as mpsum,
        ):
            # ---- constants ----
            ident = cpool.tile([128, 128], f32)
            make_identity(nc, ident[:])
            iota_i = cpool.tile([128, 128], mybir.dt.int32)
            nc.gpsimd.iota(iota_i[:], pattern=[[1, 128]], base=0, channel_multiplier=0)
            iota_f = cpool.tile([128, 128], f32)
            nc.vector.tensor_copy(iota_f[:], iota_i[:])
            w1_sb = cpool.tile([128, 2, HC + 8], f32)
            nc.sync.dma_start(out=w1_sb[:, 0], in_=w1_in[0:128])
            nc.sync.dma_start(out=w1_sb[:, 1], in_=w1_in[128:256])
            w2_sb = cpool.tile([128, 2, HC + 8], f32)
            nc.sync.dma_start(out=w2_sb[:, 0], in_=w2_in[0:128])
            nc.sync.dma_start(out=w2_sb[:, 1], in_=w2_in[128:256])
            wl1_sb = cpool.tile([C, HID], f32)
            nc.sync.dma_start(out=wl1_sb[:], in_=wl1_in[:])
            wl2_sb = cpool.tile([HID, OUT_F], f32)
            nc.sync.dma_start(out=wl2_sb[:], in_=wl2_in[:])
            bl1_sb = cpool.tile([HID, 1], f32)
            nc.sync.dma_start(out=bl1_sb[:], in_=bl1_in[:])
            bl2_sb = cpool.tile([128, OUT_F], f32)
            nc.sync.dma_start(out=bl2_sb[:], in_=bl2_in[:])
            b1_sb = cpool.tile([128, HC], f32)
            nc.sync.dma_start(out=b1_sb[:], in_=b1_in[:])
            b2_sb = cpool.tile([128, C], f32)
            nc.sync.dma_start(out=b2_sb[:], in_=b2_in[:])
            deg_sb = cpool.tile([128, TILES], f32)
            nc.sync.dma_start(out=deg_sb[:], in_=dg_in[:])

            def gemm_phase(src_dram, w_sb, xl_dst, asd_dst):
                for t in range(TILES):
                    rows = slice(t * 128, (t + 1) * 128)
                    xt = gpool.tile([128, HC], f32, tag="g_in")
                    nc.sync.dma_start(out=xt[:], in_=src_dram[rows])
                    xT = gpool.tile([128, 2, 128], f32, tag="g_T")
                    for k in range(2):
                        pst = gpsum.tile([128, 128], f32, tag="g_tp")
                        nc.tensor.transpose(pst[:], xt[:, k * 128 : (k + 1) * 128], ident[:])
                        nc.vector.tensor_copy(xT[:, k], pst[:])
                    ps = gpsum.tile([128, HC + 8], f32, tag="g_mm")
                    nc.tensor.matmul(ps[:], xT[:, 0], w_sb[:, 0], start=True, stop=False)
                    nc.tensor.matmul(ps[:], xT[:, 1], w_sb[:, 1], start=False, stop=True)
                    og = gpool.tile([128, HC + 8], f32, tag="g_out")
                    nc.vector.tensor_copy(og[:], ps[:])
                    nc.sync.dma_start(out=xl_dst[rows], in_=og[:, :HC])
                    nc.sync.dma_start(out=asd_dst[rows], in_=og[:, HC : HC + 8])

            qrr = [0]

            def edge_phase(l, sub=99):
                li = l - 1
                table = xl_fu[li]
                aspad = asd_pad[li]
                # a_d for all tiles: [128, TILES, 4]
                adr = asd_lo[li].ap().rearrange("(t d) c -> d t c", d=128)
                ad_all = cpool.tile([128, TILES, 4], f32, tag=f"ad{l}")
                nc.sync.dma_start(out=ad_all[:], in_=adr[:, :, 4:8])

                oas = oxg = 0
                for t in range(TILES):
                    Dt = D[t]
                    # --- a_s gather ---
                    was = -(-(128 * Dt + 1) // 16)
                    ias = epool.tile([128, was], i16, tag="ias")
                    nc.sync.dma_start(out=ias[:], in_=ia_in[:, oas : oas + was])
                    oas += was
                    asg = epool.tile([128, Dt + 1, 64], f32, tag="asg")
                    nc.gpsimd.dma_gather(
                        out_ap=asg[:], in_ap=aspad[BASE:, :], idxs_ap=ias[:],
                        num_idxs=128 * Dt + 1, num_idxs_reg=128 * Dt + 1,
                        elem_size=64, single_packet=False, queue_num=qrr[0] % 4,
                    )
                    qrr[0] += 1
                    # --- alpha [128, H, Dt] ---
                    alpha = spool.tile([128, H, Dt], f32, tag="alpha")
                    nc.vector.tensor_tensor(
                        out=alpha[:],
                        in0=asg[:, :Dt, 0:4].transpose([0, 2, 1]),
                        in1=ad_all[:, t].unsqueeze(2).broadcast_to([128, H, Dt]),
                        op=ADD,
                    )
                    lt1 = spool.tile([128, H, Dt], f32, tag="lt1")
                    nc.scalar.activation(lt1[:], alpha[:], CPY, scale=NEG_SLOPE)
                    nc.vector.tensor_tensor(out=alpha[:], in0=alpha[:], in1=lt1[:], op=MAXO)
                    # pad slots -> -1e30 ; padmask pm in {0,1}
                    pm = spool.tile([128, Dt], f32, tag="pm")
                    nc.vector.tensor_scalar(
                        out=pm[:], in0=iota_f[:, :Dt],
                        scalar1=deg_sb[:, t : t + 1], scalar2=None, op0=LT,
                    )
                    pb = spool.tile([128, Dt], f32, tag="pb")
                    nc.vector.tensor_scalar(
                        out=pb[:], in0=pm[:], scalar1=1.0, scalar2=1e30,
                        op0=SUB, op1=MUL,
                    )
                    nc.vector.tensor_tensor(
                        out=alpha[:], in0=alpha[:],
                        in1=pb[:].unsqueeze(1).broadcast_to([128, H, Dt]), op=ADD,
                    )
                    if sub <= 20:
                        dt_ = spool.tile([128, H], f32, tag="dbga")
                        nc.vector.tensor_copy(dt_[:], alpha[:, :, 0])
                        nc.sync.dma_start(out=out_dram[t * 128 : t * 128 + 128, 0:H], in_=dt_[:])
                        continue
                    # --- topk threshold + row max ---
                    m_all = spool.tile([128, H], f32, tag="m_all")
                    t10 = spool.tile([128, H], f32, tag="t10")
                    if Dt > K_TOP:
                        for h in range(H):
                            m8 = spool.tile([128, 8], f32, tag="m8")
                            nc.vector.max(out=m8[:], in_=alpha[:, h])
                            nc.vector.tensor_copy(m_all[:, h : h + 1], m8[:, 0:1])
                            wk = spool.tile([128, Dt], f32, tag="wk")
                            nc.vector.match_replace(
                                out=wk[:], in_to_replace=m8[:],
                                in_values=alpha[:, h], imm_value=-3e30,
                            )
                            m8b = spool.tile([128, 8], f32, tag="m8b")
                            nc.vector.max(out=m8b[:], in_=wk[:])
                            nc.vector.tensor_copy(t10[:, h : h + 1], m8b[:, 1:2])
                    else:
                        nc.vector.reduce_max(out=m_all[:], in_=alpha[:], axis=mybir.AxisListType.X)
                        nc.vector.memset(t10[:], -1e31)
                    # --- ex = exp(alpha - m) * topk_mask * padmask ---
                    ex = spool.tile([128, H, Dt], f32, tag="ex")
                    nc.vector.tensor_tensor(
                        out=ex[:], in0=alpha[:],
                        in1=m_all[:].unsqueeze(2).broadcast_to([128, H, Dt]), op=SUB,
                    )
                    nc.scalar.activation(ex[:], ex[:], EXP)
                    msk = spool.tile([128, H, Dt], f32, tag="msk")
                    nc.vector.tensor_tensor(
                        out=msk[:], in0=alpha[:],
                        in1=t10[:].unsqueeze(2).broadcast_to([128, H, Dt]), op=GE,
                    )
                    nc.vector.tensor_tensor(out=ex[:], in0=ex[:], in1=msk[:], op=MUL)
                    nc.vector.tensor_tensor(
                        out=ex[:], in0=ex[:],
                        in1=pm[:].unsqueeze(1).broadcast_to([128, H, Dt]), op=MUL,
                    )
                    # --- denom, inv ---
                    den = spool.tile([128, H], f32, tag="den")
                    nc.vector.reduce_sum(out=den[:], in_=ex[:], axis=mybir.AxisListType.X)
                    nc.vector.tensor_scalar_max(den[:], den[:], 1e-20)
                    inv = spool.tile([128, H], f32, tag="inv")
                    nc.vector.reciprocal(inv[:], den[:])
                    if l == 2:
                        nc.vector.tensor_scalar_mul(inv[:], inv[:], 1.0 / H)
                    if sub <= 21:
                        nc.sync.dma_start(out=out_dram[t * 128 : t * 128 + 128, 0:H], in_=inv[:])
                        nc.sync.dma_start(out=out_dram[t * 128 : t * 128 + 128, 4:8], in_=t10[:])
                        nc.sync.dma_start(out=out_dram[t * 128 : t * 128 + 128, 8:12], in_=m_all[:])
                        nc.sync.dma_start(out=out_dram[t * 128 : t * 128 + 128, 12:16], in_=den[:])
                        continue
                    # --- xg chunks + identity-matmul accumulate ---
                    ps = apsum.tile([128, HC], f32, tag="agg")
                    for (j0, jc) in chunks[t]:
                        wxg = -(-(128 * jc + 1) // 16)
                        ixg = epool.tile([128, wxg], i16, tag="ixg")
                        nc.sync.dma_start(out=ixg[:], in_=ix_in[:, oxg : oxg + wxg])
                        oxg += wxg
                        xg = epool.tile([128, jc + 1, HC], f32, tag="xg")
                        nc.gpsimd.dma_gather(
                            out_ap=xg[:], in_ap=table[BASE:, :], idxs_ap=ixg[:],
                            num_idxs=128 * jc + 1, num_idxs_reg=128 * jc + 1,
                            elem_size=HC, single_packet=False, queue_num=qrr[0] % 4,
                        )
                        qrr[0] += 1
                        nc.vector.tensor_tensor(
                            out=xg[:, :jc].rearrange("p j (h c) -> p j h c", h=H),
                            in0=xg[:, :jc].rearrange("p j (h c) -> p j h c", h=H),
                            in1=ex[:, :, j0 : j0 + jc].transpose([0, 2, 1])
                                .unsqueeze(3).broadcast_to([128, jc, H, C]),
                            op=MUL,
                        )
                        for j in range(jc):
                            nc.tensor.matmul(
                                ps[:], ident[:], xg[:, j],
                                start=(j0 + j == 0), stop=(j0 + j == Dt - 1),
                            )
                    # --- normalize (+ finish layer) ---
                    rows = slice(t * 128, (t + 1) * 128)
                    if l == 1:
                        o = epool.tile([128, HC], f32, tag="o1")
                        nc.vector.tensor_tensor(
                            out=o[:].rearrange("p (h c) -> p h c", h=H),
                            in0=ps[:].rearrange("p (h c) -> p h c", h=H),
                            in1=inv[:].unsqueeze(2).broadcast_to([128, H, C]), op=MUL,
                        )
                        nc.vector.tensor_tensor(out=o[:], in0=o[:], in1=b1_sb[:], op=ADD)
                        nc.sync.dma_start(out=out1_dram[rows], in_=o[:])
                    else:
                        tmp = epool.tile([128, HC], f32, tag="tmp2")
                        nc.vector.tensor_tensor(
                            out=tmp[:].rearrange("p (h c) -> p h c", h=H),
                            in0=ps[:].rearrange("p (h c) -> p h c", h=H),
                            in1=inv[:].unsqueeze(2).broadcast_to([128, H, C]), op=MUL,
                        )
                        o2 = spool.tile([128, C], f32, tag="o2")
                        nc.vector.reduce_sum(
                            out=o2[:],
                            in_=tmp[:].rearrange("p (h c) -> p c h", h=H),
                            axis=mybir.AxisListType.X,
                        )
                        nc.vector.tensor_tensor(out=o2[:], in0=o2[:], in1=b2_sb[:], op=ADD)
                        # MLP head
                        psT = mpsum.tile([C, 128], f32, tag="m_th")
                        nc.tensor.transpose(psT[:], o2[:], ident[:])
                        o2T = spool.tile([C, 128], f32, tag="o2T")
                        nc.vector.tensor_copy(o2T[:], psT[:])
                        psh = mpsum.tile([HID, 128], f32, tag="m_th")
                        nc.tensor.matmul(psh[:], wl1_sb[:], o2T[:], start=True, stop=True)
                        rh = spool.tile([HID, 128], f32, tag="rh")
                        nc.scalar.activation(rh[:], psh[:], RELU, bias=bl1_sb[:])
                        pso = mpsum.tile([OUT_F, 128], f32, tag="m_of")
                        nc.tensor.matmul(pso[:], wl2_sb[:], rh[:], start=True, stop=True)
                        po = spool.tile([OUT_F, 128], f32, tag="po")
                        nc.vector.tensor_copy(po[:], pso[:])
                        psf = mpsum.tile([128, OUT_F], f32, tag="m_of")
                        nc.tensor.transpose(psf[:], po[:], ident[:OUT_F, :OUT_F])
                        of = spool.tile([128, OUT_F], f32, tag="of")
                        nc.vector.tensor_tensor(out=of[:], in0=psf[:], in1=bl2_sb[:], op=ADD)
                        nc.sync.dma_start(out=out_dram[rows], in_=of[:])

            def allgather(l):
                li = l - 1
                nc.gpsimd.collective_compute(
                    "AllGather", mybir.AluOpType.bypass,
                    replica_groups=[list(range(N_CORES))],
                    ins=[xl_sh[li].ap().opt()], outs=[xl_fu[li].ap().opt()],
                )
                nc.gpsimd.collective_compute(
                    "AllGather", mybir.AluOpType.bypass,
                    replica_groups=[list(range(N_CORES))],
                    ins=[asd_lo[li].ap().opt()], outs=[asd_fu[li].ap().opt()],
                )
                nc.sync.dma_start(out=asd_pad[li][:, 0:8], in_=asd_fu[li][:])

            for _rep in range(repeat):
                gemm_phase(x_in, w1_sb, xl_sh[0], asd_lo[0])
                if stage >= 1:
                    allgather(1)
                if stage >= 2:
                    edge_phase(1, sub=sub)
                if stage >= 3:
                    gemm_phase(out1_dram, w2_sb, xl_sh[1], asd_lo[1])
                    allgather(2)
                if stage >= 4:
                    edge_phase(2)
            if stage < 4 and sub > 21:
                # debug readout so nothing is dead code
                dbg_src = out1_dram if stage >= 2 else xl_sh[0]
                if stage == 3:
                    dbg_src = xl_sh[1]
                for t in range(TILES):
                    rows = slice(t * 128, (t + 1) * 128)
                    dtile = spool.tile([128, OUT_F], f32, tag="dbg")
                    nc.sync.dma_start(out=dtile[:], in_=dbg_src[rows, 0:OUT_F])
                    nc.sync.dma_start(out=out_dram[rows], in_=dtile[:])
                # also touch asd so it isn't dead
                dp = spool.tile([128, 8], f32, tag="dbgp")
                asrc = asd_pad[0] if stage >= 1 else asd_lo[0]
                nc.sync.dma_start(out=dp[:], in_=asrc[0:128, 0:8])
                nc.sync.dma_start(out=out_dram[0:128, 0:8], in_=dp[:])

    nc.compile()
    return nc


def _make_in_maps(consts, per_core, b1, b2, bl1, bl2):
    b1 = np.asarray(b1, np.float32)
    b2 = np.asarray(b2, np.float32)
    bl1 = np.asarray(bl1, np.float32)
    bl2 = np.asarray(bl2, np.float32)
    shared = dict(
        W1T_ext=consts["W1T_ext"], W2T_ext=consts["W2T_ext"],
        Wl1T=consts["Wl1T"], Wl2T=consts["Wl2T"],
        bl1_col=np.ascontiguousarray(bl1[:, None]),
        bl2_rep=np.tile(bl2[None, :], (128, 1)),
        b1_rep=np.tile(b1[None, :], (128, 1)),
        b2_rep=np.tile(b2[None, :], (128, 1)),
    )
    return [
        dict(
            shared,
            x_shard=np.ascontiguousarray(per_core["x_shard"][c]),
            idx_as=np.ascontiguousarray(per_core["idx_as"][c]),
            idx_xg=np.ascontiguousarray(per_core["idx_xg"][c]),
            degf=np.ascontiguousarray(per_core["degf"][c]),
        )
        for c in range(N_CORES)
    ]


def _assemble(results, node_of):
    out = np.empty((N, OUT_F), np.float32)
    for c in range(N_CORES):
        out[node_of[c, :SH]] = results[c]["out"][:SH]
    return out


def kernel(x, W1, att_s1, att_d1, b1, W2, att_s2, att_d2, b2,
           Wl1, bl1, Wl2, bl2, edge_index):
    from concourse.bass_utils import run_bass_kernel_spmd

    meta, consts, per_core, node_of = _prep(
        x, W1, att_s1, att_d1, W2, att_s2, att_d2, Wl1, Wl2, edge_index
    )
    nc = build_gnn(meta)
    in_maps = _make_in_maps(consts, per_core, b1, b2, bl1, bl2)
    res = run_bass_kernel_spmd(nc, in_maps, core_ids=list(range(N_CORES)))
    return _assemble(res.results, node_of)

